# revision 10
# baseline (speedup 1.0000x reference)
"""Trainium2 Bass kernel for nn_ChemModel (DMPNN-style message-passing GNN).

Self-contained: call kernel(**inputs) with the full (unsharded) inputs from
setup_inputs(); returns the full [N_GRAPHS, 1] float32 output.

Strategy (8 NeuronCores, SPMD — one program, per-core data):
  * Nodes/slots sharded by dst owner in contiguous ranges of N/8. Only h rows
    [0, N) evolve; rows >= N keep h0 and are recomputed during the final
    aggregation from host-packed [x[src]|edge_attr] columns (no gather).
  * Persistent transposed state hA_T [128h x SHP] in SBUF.
  * Message tables live in DRAM with a partition-major row order
    (row = core*SHP + (slot%128)*NB + slot//128) so the local shard is
    written with ONE contiguous DMA, then AllGathered (bf16).
  * Per-edge messages are fetched with a SINGLE dma_gather pass directly in
    dst-sorted order: edges are grouped by table-row % 4 and gathered with
    elem_step=4*H (int16 indices then cover the whole 100K-row table).
  * Scatter-add is one-hot matmul into sliding 256-slot PSUM windows with a
    host-computed, core-uniform window schedule (max over cores); padding
    edges carry dloc=-1 and self-neutralize in the one-hot.
  * Final phase: h-final rows for edge ids < N come from one more
    table+gather pass; edge ids >= N recompute h0 via matmul from host-packed
    inputs, overlapped with the collective + gathers.
"""
import math
import numpy as np
import ml_dtypes

import concourse.bass as bass
from concourse import bacc
import concourse.mybir as mybir
import concourse.tile as tile
from concourse.bass_utils import run_bass_kernel_spmd
from concourse import library_config

P = 128
H = 128
NCORES = 8
GIDX_N = 2048              # indices per dma_gather instruction
GCH = GIDX_N // P          # chunks per gather instruction (16)
ICOL = GIDX_N // 16        # idx columns per instruction in wrapped layout
WSL = 256                  # scatter window width in slots (2 blocks)
NRES = 4                   # residue groups (table row % 4)
JMAX = 8                   # max windows spanned by one chunk
F32 = mybir.dt.float32
BF16 = mybir.dt.bfloat16
I16 = mybir.dt.int16
BF16NP = ml_dtypes.bfloat16


def _relu():
    return mybir.ActivationFunctionType.Relu


def _wrap_idx16(flat):
    """[n] int array -> [128, n//16] int16 wrapped layout."""
    n = flat.shape[0]
    assert n % 16 == 0
    w = flat.reshape(n // 16, 16).T.astype(np.int16)
    return np.tile(w, (8, 1))


class _Plan:
    pass


def _build_stream_groups(rows_key, dloc, extra=None):
    """Split one core's edge stream into NRES residue groups sorted by dloc.

    Returns per group dict with idx ((row - r) // 4), dloc, and optionally a
    sorted copy of `extra` (per-edge payload columns [n, width])."""
    out = []
    for r in range(NRES):
        m = (rows_key % NRES) == r
        dl = dloc[m]
        rw = rows_key[m]
        o = np.argsort(dl, kind="stable")
        g = {"idx": (rw[o] - r) // NRES, "dloc": dl[o]}
        if extra is not None:
            g["extra"] = extra[m][o]
        out.append(g)
    return out


def _chunk_minmax(dloc_pad):
    """dloc_pad: [nch, P] with -1 pads -> per-chunk (min, max) over real
    entries; (inf, -inf) when empty."""
    real = dloc_pad >= 0
    mn = np.where(real, dloc_pad, np.inf).min(axis=1)
    mx = np.where(real, dloc_pad, -np.inf).max(axis=1)
    return mn, mx


def _host_prep(x, edge_index, edge_attr, batch, depth, G):
    N, E = x.shape[0], edge_index.shape[1]
    src = edge_index[0].astype(np.int64)
    dst = edge_index[1].astype(np.int64)
    batch = batch.astype(np.int64)
    x = np.asarray(x, np.float32)
    ea = np.asarray(edge_attr, np.float32)

    assert N % NCORES == 0
    NSH = N // NCORES
    NB = math.ceil(NSH / P)
    SHP = NB * P
    TBL = NCORES * SHP
    assert TBL % NRES == 0 and TBL // NRES <= 32512
    assert SHP % WSL == 0
    NW = SHP // WSL

    plan = _Plan()
    plan.N, plan.E, plan.G = N, E, G
    plan.NSH, plan.NB, plan.SHP, plan.TBL = NSH, NB, SHP, TBL
    plan.NW = NW
    plan.depth = int(depth)
    plan.GW = min(512, G)

    def row_of(n):
        s = n % NSH
        return (n // NSH) * SHP + (s % P) * NB + s // P

    row_src = row_of(src)

    # per-core edge partitions by dst owner
    core_of = dst // NSH
    per_core_loop = []   # all E edges (loop + used again conceptually)
    per_core_f0 = []     # edge ids < N  (gather h_final rows)
    per_core_f1 = []     # edge ids >= N (recompute h0)
    eids = np.arange(E)
    for k in range(NCORES):
        m = core_of == k
        ek = eids[m]
        dl = dst[m] - k * NSH
        per_core_loop.append(
            _build_stream_groups(row_src[ek], dl))
        m0 = ek < N
        per_core_f0.append(
            _build_stream_groups(row_of(ek[m0]), dl[m0]))
        ek1 = ek[m0 == False]  # noqa: E712
        xe = np.concatenate([x[src[ek1]], ea[ek1]], axis=1)  # [n1, 7]
        dl1 = dl[m0 == False]  # noqa: E712
        o = np.argsort(dl1, kind="stable")
        per_core_f1.append({"dloc": dl1[o], "extra": xe[o]})

    def finish_phase(groups_by_core, unit):
        """groups_by_core: [NCORES][NRES] dicts. unit: pad granularity
        (GIDX_N for gather streams). Produces uniform instruction counts,
        per-core idx/dloc tables and the shared window schedule."""
        ph = {}
        n_instr = []
        for g in range(NRES):
            mx = max(len(groups_by_core[k][g]["idx"]) for k in range(NCORES))
            n_instr.append(max(1, math.ceil(mx / unit)))
        ph["n_instr"] = n_instr
        nch_g = [ni * (unit // P) for ni in n_instr]
        ph["nch_g"] = nch_g
        # per-core padded dloc [group][core][nch, P]
        dl_pad = []
        for g in range(NRES):
            percore = []
            for k in range(NCORES):
                dl = groups_by_core[k][g]["dloc"]
                buf = np.full(nch_g[g] * P, -1.0, np.float64)
                buf[:len(dl)] = dl
                percore.append(buf.reshape(nch_g[g], P))
            dl_pad.append(percore)
        # window schedule (uniform across cores)
        sched = []
        for g in range(NRES):
            mns = np.full((NCORES, nch_g[g]), np.inf)
            mxs = np.full((NCORES, nch_g[g]), -np.inf)
            for k in range(NCORES):
                mns[k], mxs[k] = _chunk_minmax(dl_pad[g][k])
            mn = mns.min(axis=0)
            mx = mxs.max(axis=0)
            wfirst = np.where(np.isfinite(mn), mn // WSL, -1).astype(np.int64)
            wlast = np.where(np.isfinite(mx), mx // WSL, -2).astype(np.int64)
            span = (wlast - wfirst + 1).clip(min=0)
            assert span.max(initial=0) <= JMAX, f"window span {span.max()}"
            wl = {}
            for i in range(nch_g[g]):
                for w in range(wfirst[i], wlast[i] + 1):
                    wl.setdefault(w, []).append(i)
            # per chunk: list of (w, j, start, stop)
            chunks = []
            for i in range(nch_g[g]):
                lst = []
                for w in range(wfirst[i], wlast[i] + 1):
                    lst.append((int(w), int(w - wfirst[i]),
                                wl[w][0] == i, wl[w][-1] == i))
                chunks.append(lst)
            sched.append({"chunks": chunks, "wfirst": wfirst})
        ph["sched"] = sched
        # per-core relative dloc columns [128, sum(nch_g)] f32
        dloc_cols = []
        for k in range(NCORES):
            cols = []
            for g in range(NRES):
                wf = sched[g]["wfirst"]
                rel = dl_pad[g][k] - (wf[:, None] * WSL)
                rel[dl_pad[g][k] < 0] = -1.0
                cols.append(rel.reshape(nch_g[g], P).T)
            dloc_cols.append(
                np.ascontiguousarray(np.concatenate(cols, axis=1)
                                     .astype(np.float32)))
        ph["dloc"] = dloc_cols
        # per-core wrapped idx tables [128, sum(n_instr)*ICOL] i16
        idx_cols = []
        for k in range(NCORES):
            cols = []
            for g in range(NRES):
                idx = groups_by_core[k][g]["idx"]
                buf = np.zeros(n_instr[g] * unit, np.int64)  # pad -> row 0
                buf[:len(idx)] = idx
                for ii in range(n_instr[g]):
                    cols.append(_wrap_idx16(buf[ii * unit:(ii + 1) * unit]))
            idx_cols.append(np.ascontiguousarray(np.concatenate(cols, axis=1)))
        ph["ga"] = idx_cols
        return ph

    plan.loop = finish_phase(per_core_loop, GIDX_N)
    plan.f0 = finish_phase(per_core_f0, GIDX_N)

    # fin stream 1 (no gather): chunk/window schedule + packed [7, C1*P] cols
    C1 = max(1, max(math.ceil(len(per_core_f1[k]["dloc"]) / P)
                    for k in range(NCORES)))
    C1 = math.ceil(C1 / GCH) * GCH   # round to full 2048-col batches
    plan.C1 = C1
    dl_pad1, mns, mxs = [], np.full((NCORES, C1), np.inf), \
        np.full((NCORES, C1), -np.inf)
    for k in range(NCORES):
        dl = per_core_f1[k]["dloc"]
        buf = np.full(C1 * P, -1.0, np.float64)
        buf[:len(dl)] = dl
        dl_pad1.append(buf.reshape(C1, P))
        mns[k], mxs[k] = _chunk_minmax(dl_pad1[k])
    mn, mx = mns.min(axis=0), mxs.max(axis=0)
    wfirst = np.where(np.isfinite(mn), mn // WSL, -1).astype(np.int64)
    wlast = np.where(np.isfinite(mx), mx // WSL, -2).astype(np.int64)
    assert (wlast - wfirst + 1).clip(min=0).max(initial=0) <= JMAX
    wl = {}
    for i in range(C1):
        for w in range(wfirst[i], wlast[i] + 1):
            wl.setdefault(w, []).append(i)
    chunks1 = []
    for i in range(C1):
        lst = []
        for w in range(wfirst[i], wlast[i] + 1):
            lst.append((int(w), int(w - wfirst[i]),
                        wl[w][0] == i, wl[w][-1] == i))
        chunks1.append(lst)
    plan.f1_sched = chunks1
    plan.f1_dloc, plan.f1_xe = [], []
    for k in range(NCORES):
        rel = dl_pad1[k] - (wfirst[:, None] * WSL)
        rel[dl_pad1[k] < 0] = -1.0
        plan.f1_dloc.append(np.ascontiguousarray(
            rel.reshape(C1, P).T.astype(np.float32)))
        xe = np.zeros((C1 * P, 7), np.float32)
        n1 = len(per_core_f1[k]["dloc"])
        xe[:n1] = per_core_f1[k]["extra"]
        plan.f1_xe.append(np.ascontiguousarray(xe.T.astype(BF16NP)))

    # init: [7, SHP] = [x[src[slot]], ea[slot]] per core
    plan.xe_init = []
    for k in range(NCORES):
        sl = np.arange(k * NSH, (k + 1) * NSH)
        xe = np.zeros((SHP, 7), np.float32)
        xe[:NSH, :4] = x[src[sl]]
        xe[:NSH, 4:] = ea[sl]
        plan.xe_init.append(np.ascontiguousarray(xe.T.astype(BF16NP)))

    # node features for the final W_a matmul + pooling info
    plan.xT = []
    plan.batchloc = []
    plan.g_bases = []
    for k in range(NCORES):
        xs = np.zeros((SHP, 4), np.float32)
        xs[:NSH] = x[k * NSH:(k + 1) * NSH]
        plan.xT.append(np.ascontiguousarray(xs.T.astype(BF16NP)))
        gb = int(batch[k * NSH])
        ge = int(batch[(k + 1) * NSH - 1])
        assert ge - gb < plan.GW, f"graph span {ge - gb} >= {plan.GW}"
        plan.g_bases.append(gb)
        bl = np.full((SHP,), -1.0, np.float32)
        bl[:NSH] = batch[k * NSH:(k + 1) * NSH] - gb
        plan.batchloc.append(
            np.ascontiguousarray(bl.reshape(NB, P).T))

    plan.nA_L = sum(plan.loop["n_instr"])
    plan.nA_F = sum(plan.f0["n_instr"])
    plan.ncols_L = sum(plan.loop["nch_g"])
    plan.ncols_F = sum(plan.f0["nch_g"])
    return plan


# ----------------------------------------------------------------------------
# device kernel
# ----------------------------------------------------------------------------

def _build(plan, split=True):
    NB, SHP, TBL, NW = plan.NB, plan.SHP, plan.TBL, plan.NW
    G, GW = plan.G, plan.GW
    depth = plan.depth
    TBL4 = TBL // NRES

    nc = bacc.Bacc(num_devices=NCORES)

    def din(name, shape, dt=F32):
        return nc.declare_dram_parameter(name, list(shape), dt, isOutput=False)

    WmT = din("WmT", [H, H], BF16)
    Wi7T = din("Wi7T", [7, H], BF16)
    WaxT = din("WaxT", [4, H], BF16)
    WahT = din("WahT", [H, H], BF16)
    W1T = din("W1T", [H, 4 * H])
    W2T = din("W2T", [4 * H, H])
    WlastT = din("WlastT", [H, 1])
    b1r = din("b1r", [H, 4])
    b2r = din("b2r", [H, 1])
    blast = din("blast", [1, 1])
    iotaWJ = din("iotaWJ", [P, JMAX * WSL])
    iotaG = din("iotaG", [P, GW])
    ident = din("ident", [P, P])
    xeI_in = din("xeI", [7, SHP], BF16)
    xeF_in = din("xeF", [7, plan.C1 * P], BF16)
    xT_in = din("xT", [4, SHP], BF16)
    batchloc_in = din("batchloc", [P, NB])
    gaL_in = din("gaL", [P, plan.nA_L * ICOL], I16)
    gaF_in = din("gaF", [P, plan.nA_F * ICOL], I16)
    dlocL_in = din("dlocL", [P, plan.ncols_L])
    dlocF_in = din("dlocF", [P, plan.ncols_F])
    dlocF1_in = din("dlocF1", [P, plan.C1])

    out_ext = nc.declare_dram_parameter("out", [G, 1], F32, isOutput=True)

    RG = list(range(NCORES))

    with tile.TileContext(nc) as tc:
        nc.gpsimd.load_library(library_config.mlp)
        with (
            tc.tile_pool(name="cp", bufs=1) as cp,
            tc.tile_pool(name="sb", bufs=3) as sb,
            tc.tile_pool(name="ps", bufs=2, space="PSUM") as ps,
            tc.tile_pool(name="dr", bufs=1, space="DRAM") as dr,
        ):
            def cload(name, src_t):
                tl = cp.tile([src_t.shape[0], src_t.shape[1]], src_t.dtype,
                             name=name)
                nc.sync.dma_start(out=tl[:], in_=src_t[:, :])
                return tl

            WmT_s = cload("WmT_s", WmT)
            Wi7T_s = cload("Wi7T_s", Wi7T)
            WaxT_s = cload("WaxT_s", WaxT)
            WahT_s = cload("WahT_s", WahT)
            W1T_s = cload("W1T_s", W1T)
            W2T_f = []
            for f in range(4):
                tl = cp.tile([P, H], F32, name=f"W2T_{f}")
                nc.sync.dma_start(out=tl[:], in_=W2T[f * P:(f + 1) * P, :])
                W2T_f.append(tl)
            WlastT_s = cload("WlastT_s", WlastT)
            b1r_s = cload("b1r_s", b1r)
            b2r_s = cload("b2r_s", b2r)
            blast_s = cload("blast_s", blast)
            iotaWJ_s = cload("iotaWJ_s", iotaWJ)
            iotaG_s = cload("iotaG_s", iotaG)
            ident_s = cload("ident_s", ident)
            batchloc_s = cload("batchloc_s", batchloc_in)
            gaL_s = cload("gaL_s", gaL_in)
            gaF_s = cload("gaF_s", gaF_in)
            dlocL_s = cload("dlocL_s", dlocL_in)
            dlocF_s = cload("dlocF_s", dlocF_in)
            dlocF1_s = cload("dlocF1_s", dlocF1_in)

            hA_T = cp.tile([P, SHP], F32, name="hA_T")
            nacc_T = cp.tile([P, SHP], F32, name="nacc_T")
            nc.vector.memset(nacc_T[:], 0.0)
            tab_tile = cp.tile([P, NB, H], BF16, name="tab_tile")
            gfull = cp.tile([P, G], F32, name="gfull")

            loc = dr.tile([SHP, H], BF16, name="loc")
            tbls = [dr.tile([TBL, H], BF16, name=f"tbl{i}",
                            addr_space="Shared") for i in range(depth + 1)]
            gwin_local = dr.tile([P, GW], F32, name="gwin_local")
            gwin_all = dr.tile([NCORES * P, GW], F32, name="gwin_all",
                               addr_space="Shared")

            def allgather(local, table):
                nc.gpsimd.collective_compute(
                    "AllGather", mybir.AluOpType.bypass,
                    replica_groups=[RG], ins=[local[:]], outs=[table[:]])

            def gather(dst_ap, tbl_t, res, idx_sb, inst_col):
                base = tbl_t[:]
                in_ap = bass.AP(base.tensor, base.offset + res * H,
                                [[NRES * H, TBL4], [1, H]])
                nc.gpsimd.dma_gather(
                    out_ap=dst_ap, in_ap=in_ap,
                    idxs_ap=idx_sb[:, inst_col * ICOL:(inst_col + 1) * ICOL],
                    num_idxs=GIDX_N, num_idxs_reg=GIDX_N,
                    elem_size=H, elem_step=NRES * H, single_packet=False)

            # ------------- scatter machinery (shared schedule walker) -------
            def scatter_chunks(chunk_iter, acc):
                """chunk_iter yields (msg_tile, cslice, dloc_col, wlist).
                One-hot matmul per (chunk, window) into a fresh PSUM tile,
                immediately added into `acc` [P, SHP]."""
                for msg, csl, dcol, wlist in chunk_iter:
                    for (w, j, st, sp) in wlist:
                        oh = sb.tile([P, WSL], BF16, name="oh", tag="oh",
                                     bufs=4)
                        nc.vector.tensor_tensor(
                            out=oh[:],
                            in0=dcol.to_broadcast([P, WSL]),
                            in1=iotaWJ_s[:, j * WSL:(j + 1) * WSL],
                            op=mybir.AluOpType.is_equal)
                        pw = ps.tile([P, WSL], F32, name="pw", tag="pacc",
                                     space="PSUM", bufs=3)
                        nc.tensor.matmul(out=pw[:], lhsT=msg[csl],
                                         rhs=oh[:], start=True, stop=True)
                        a = acc[:, w * WSL:(w + 1) * WSL]
                        nc.vector.tensor_add(out=a, in0=a, in1=pw[:])

            def gather_phase_chunks(ph, dloc_s, ga_s, tbl_t):
                """Generator of scatter_chunks items for a gather phase.
                Gathers are emitted with lookahead 2 so that tile-pool buffer
                reuse (bufs=3) never outruns already-emitted consumers."""
                LA = 2
                icol = 0
                col0 = 0
                for g in range(NRES):
                    sched = ph["sched"][g]["chunks"]
                    nin = ph["n_instr"][g]
                    tiles = [None] * nin

                    def emit(ii, icol0=icol):
                        gt = sb.tile([P, GCH, H], BF16, name="gt", tag="gat")
                        gather(gt[:], tbl_t, g, ga_s, icol0 + ii)
                        tiles[ii] = gt
                    for ii in range(min(LA, nin)):
                        emit(ii)
                    for ii in range(nin):
                        if ii + LA < nin:
                            emit(ii + LA)
                        for c in range(GCH):
                            i = ii * GCH + c
                            wlist = sched[i]
                            if not wlist:
                                continue
                            yield (tiles[ii], np.s_[:, c, :],
                                   dloc_s[:, col0 + i:col0 + i + 1], wlist)
                    icol += nin
                    col0 += ph["nch_g"][g]

            # ---------------- init ----------------
            for bi in range(math.ceil(NB / GCH)):
                b0 = bi * GCH
                b1_ = min(NB, b0 + GCH)
                xei = sb.tile([7, GIDX_N], BF16, name="xei", tag="xet",
                              bufs=2)
                nc.sync.dma_start(
                    out=xei[:, :(b1_ - b0) * P],
                    in_=xeI_in[:, b0 * P:b1_ * P])
                for b in range(b0, b1_):
                    pi = ps.tile([P, H], F32, name="pi", tag="ptmp",
                                 space="PSUM")
                    nc.tensor.matmul(out=pi[:], lhsT=Wi7T_s[:],
                                     rhs=xei[:, (b - b0) * P:(b - b0 + 1) * P],
                                     start=True, stop=True)
                    nc.scalar.activation(out=hA_T[:, b * P:(b + 1) * P],
                                         in_=pi[:], func=_relu())

            # ---------------- message-passing iterations ----------------
            for it in range(depth):
                # mA = relu(W_m h) -> bf16 table tile -> DRAM -> AllGather
                for b in range(NB):
                    hb = sb.tile([P, P], BF16, name="hb", tag="hb")
                    nc.vector.tensor_copy(out=hb[:],
                                          in_=hA_T[:, b * P:(b + 1) * P])
                    pm = ps.tile([P, H], F32, name="pm", tag="ptmp",
                                 space="PSUM")
                    nc.tensor.matmul(out=pm[:], lhsT=hb[:], rhs=WmT_s[:],
                                     start=True, stop=True)
                    nc.scalar.activation(out=tab_tile[:, b, :], in_=pm[:],
                                         func=_relu())
                nc.sync.dma_start(
                    out=loc[:, :].rearrange("(p b) h -> p (b h)", p=P),
                    in_=tab_tile[:])
                allgather(loc, tbls[it])
                scatter_chunks(
                    gather_phase_chunks(plan.loop, dlocL_s, gaL_s, tbls[it]),
                    hA_T)

            # ---------------- final aggregation ----------------
            # h_final table (transposed state) -> DRAM -> AllGather
            for b in range(NB):
                pt = ps.tile([P, H], F32, name="pt", tag="ptmp", space="PSUM")
                nc.tensor.transpose(out=pt[:], in_=hA_T[:, b * P:(b + 1) * P],
                                    identity=ident_s[:])
                nc.vector.tensor_copy(out=tab_tile[:, b, :], in_=pt[:])
            nc.sync.dma_start(
                out=loc[:, :].rearrange("(p b) h -> p (b h)", p=P),
                in_=tab_tile[:])
            allgather(loc, tbls[depth])

            # stream 1: h0 recompute for edge ids >= N (overlaps AG + gathers)
            def f1_chunks():
                nbatch = plan.C1 // GCH
                for bi in range(nbatch):
                    xet = sb.tile([7, GIDX_N], BF16, name="xet", tag="xet",
                                  bufs=2)
                    nc.sync.dma_start(
                        out=xet[:],
                        in_=xeF_in[:, bi * GIDX_N:(bi + 1) * GIDX_N])
                    for c in range(GCH):
                        i = bi * GCH + c
                        wlist = plan.f1_sched[i]
                        if not wlist:
                            continue
                        ph0 = ps.tile([P, H], F32, name="ph0", tag="ptmp",
                                      space="PSUM")
                        nc.tensor.matmul(out=ph0[:],
                                         lhsT=xet[:, c * P:(c + 1) * P],
                                         rhs=Wi7T_s[:], start=True, stop=True)
                        msg = sb.tile([P, H], BF16, name="msg", tag="msg",
                                      bufs=4)
                        nc.scalar.activation(out=msg[:], in_=ph0[:],
                                             func=_relu())
                        yield (msg, np.s_[:, :],
                               dlocF1_s[:, i:i + 1], wlist)

            # emit: f0 gathers first (Pool/DMA), then f1 (PE) runs under them,
            # then f0 scatter consumes the gathered tiles.
            f0_iter = gather_phase_chunks(plan.f0, dlocF_s, gaF_s, tbls[depth])
            scatter_chunks(f1_chunks(), nacc_T)
            scatter_chunks(f0_iter, nacc_T)

            # ---------------- node_emb + pooling ----------------
            gps = ps.tile([P, GW], F32, name="gps", tag="gps", space="PSUM",
                          bufs=1)
            for b in range(NB):
                if b % GCH == 0:
                    b0 = b
                    b1_ = min(NB, b0 + GCH)
                    xtt = sb.tile([4, GIDX_N], BF16, name="xtt", tag="xet",
                                  bufs=2)
                    nc.sync.dma_start(
                        out=xtt[:, :(b1_ - b0) * P],
                        in_=xT_in[:, b0 * P:b1_ * P])
                nb16 = sb.tile([P, P], BF16, name="nb16", tag="hb")
                nc.vector.tensor_copy(out=nb16[:],
                                      in_=nacc_T[:, b * P:(b + 1) * P])
                p2 = ps.tile([P, H], F32, name="p2", tag="ptmp", space="PSUM")
                nc.tensor.matmul(out=p2[:], lhsT=nb16[:],
                                 rhs=WahT_s[:], start=True, stop=False)
                nc.tensor.matmul(out=p2[:],
                                 lhsT=xtt[:, (b - b0) * P:(b - b0 + 1) * P],
                                 rhs=WaxT_s[:], start=False, stop=True)
                ne2 = sb.tile([P, H], BF16, name="ne2", tag="msg", bufs=4)
                nc.scalar.activation(out=ne2[:], in_=p2[:], func=_relu())
                ohg = sb.tile([P, GW], BF16, name="ohg", tag="ohg")
                nc.vector.tensor_tensor(
                    out=ohg[:],
                    in0=batchloc_s[:, b:b + 1].to_broadcast([P, GW]),
                    in1=iotaG_s[:], op=mybir.AluOpType.is_equal)
                nc.tensor.matmul(out=gps[:], lhsT=ne2[:], rhs=ohg[:],
                                 start=(b == 0), stop=(b == NB - 1))

            tgw = sb.tile([P, GW], F32, name="tgw", tag="ohg")
            nc.vector.tensor_copy(out=tgw[:], in_=gps[:])
            nc.sync.dma_start(out=gwin_local[:, :], in_=tgw[:])
            allgather(gwin_local, gwin_all)
            nc.vector.memset(gfull[:], 0.0)
            for j in range(NCORES):
                wj = min(GW, G - plan.g_bases[j])
                tw = sb.tile([P, GW], F32, name="twj", tag="ohg")
                nc.sync.dma_start(out=tw[:], in_=gwin_all[j * P:(j + 1) * P, :])
                nc.vector.tensor_add(
                    out=gfull[:, plan.g_bases[j]:plan.g_bases[j] + wj],
                    in0=gfull[:, plan.g_bases[j]:plan.g_bases[j] + wj],
                    in1=tw[:, :wj])

            # ---------------- FFN (replicated on all cores) ----------------
            NGC = math.ceil(G / 512)
            for gc in range(NGC):
                g0, g1 = gc * 512, min((gc + 1) * 512, G)
                pz2 = ps.tile([P, 512], F32, name="pz2", tag="pz2",
                              space="PSUM", bufs=1)
                for f in range(4):
                    pz = ps.tile([P, 512], F32, name="pz", tag="ptmp",
                                 space="PSUM")
                    nc.tensor.matmul(out=pz[:, :g1 - g0],
                                     lhsT=W1T_s[:, f * P:(f + 1) * P],
                                     rhs=gfull[:, g0:g1], start=True, stop=True)
                    z1 = sb.tile([P, 512], F32, name="z1", tag="z1", bufs=2)
                    nc.scalar.activation(out=z1[:, :g1 - g0],
                                         in_=pz[:, :g1 - g0], func=_relu(),
                                         bias=b1r_s[:, f:f + 1])
                    nc.tensor.matmul(out=pz2[:, :g1 - g0], lhsT=W2T_f[f][:],
                                     rhs=z1[:, :g1 - g0], start=(f == 0),
                                     stop=(f == 3))
                z2 = sb.tile([P, 512], F32, name="z2", tag="z1", bufs=2)
                nc.vector.tensor_add(
                    out=z2[:, :g1 - g0], in0=pz2[:, :g1 - g0],
                    in1=b2r_s[:, 0:1].to_broadcast([P, g1 - g0]))
                po = ps.tile([1, 512], F32, name="po", tag="ptmp",
                             space="PSUM")
                nc.tensor.matmul(out=po[:, :g1 - g0], lhsT=WlastT_s[:],
                                 rhs=z2[:, :g1 - g0], start=True, stop=True)
                oc = sb.tile([1, 512], F32, name="oc", tag="oc", bufs=2)
                nc.vector.tensor_add(
                    out=oc[:, :g1 - g0], in0=po[:, :g1 - g0],
                    in1=blast_s[0:1, 0:1].to_broadcast([1, g1 - g0]))
                nc.sync.dma_start(out=out_ext[g0:g1, :], in_=oc[:, :g1 - g0])

    nc.compile()
    if split:
        _split_excess_waits(nc)
    return nc


def _split_excess_waits(nc, max_waits=1):
    k = 0
    for f in nc.m.functions:
        for bb in f.blocks:
            new = []
            for ins in bb.instructions:
                si = ins.sync_info
                if si is not None and len(si.on_wait) > max_waits:
                    waits = list(si.on_wait)
                    for w in waits[:-max_waits]:
                        nop = mybir.InstNoOp(name=f"I-waitsplit-{k}",
                                             engine=ins.engine)
                        k += 1
                        nop.sync_info = mybir.SyncInfo(on_wait=[w],
                                                       on_update=[])
                        new.append(nop)
                    si.on_wait = waits[-max_waits:]
                new.append(ins)
            bb.instructions = new
    return k


# ----------------------------------------------------------------------------
# inputs
# ----------------------------------------------------------------------------

def _in_maps(plan, weights):
    com = {
        "WmT": np.ascontiguousarray(weights["W_m"].T).astype(BF16NP),
        "Wi7T": np.ascontiguousarray(weights["W_i"].T).astype(BF16NP),
        "WaxT": np.ascontiguousarray(weights["W_a"][:, :4].T).astype(BF16NP),
        "WahT": np.ascontiguousarray(weights["W_a"][:, 4:].T).astype(BF16NP),
        "W1T": np.ascontiguousarray(weights["W1"].T),
        "W2T": np.ascontiguousarray(weights["W2"].T),
        "WlastT": np.ascontiguousarray(weights["W_last"].T),
        "b1r": np.ascontiguousarray(weights["b1"].reshape(4, H).T),
        "b2r": weights["b2"].reshape(H, 1).copy(),
        "blast": weights["b_last"].reshape(1, 1).copy(),
        "iotaWJ": np.tile(np.arange(JMAX * WSL, dtype=np.float32), (P, 1)),
        "iotaG": np.tile(np.arange(plan.GW, dtype=np.float32), (P, 1)),
        "ident": np.eye(P, dtype=np.float32),
    }
    maps = []
    for k in range(NCORES):
        m = dict(com)
        m["xeI"] = plan.xe_init[k]
        m["xeF"] = plan.f1_xe[k]
        m["xT"] = plan.xT[k]
        m["batchloc"] = plan.batchloc[k]
        m["gaL"] = plan.loop["ga"][k]
        m["gaF"] = plan.f0["ga"][k]
        m["dlocL"] = plan.loop["dloc"][k]
        m["dlocF"] = plan.f0["dloc"][k]
        m["dlocF1"] = plan.f1_dloc[k]
        maps.append(m)
    return maps


def _prep_all(x, edge_index, edge_attr, batch, depth, weights, G):
    plan = _host_prep(np.asarray(x, np.float32), np.asarray(edge_index),
                      np.asarray(edge_attr, np.float32), np.asarray(batch),
                      int(depth), G)
    maps = _in_maps(plan, weights)
    return plan, maps


def kernel(x, edge_index, edge_attr, batch, depth,
           W_i, W_m, W_a, W1, b1, W2, b2, W_last, b_last):
    weights = {
        "W_i": np.asarray(W_i, np.float32), "W_m": np.asarray(W_m, np.float32),
        "W_a": np.asarray(W_a, np.float32), "W1": np.asarray(W1, np.float32),
        "b1": np.asarray(b1, np.float32), "W2": np.asarray(W2, np.float32),
        "b2": np.asarray(b2, np.float32),
        "W_last": np.asarray(W_last, np.float32),
        "b_last": np.asarray(b_last, np.float32),
    }
    G = 2048
    plan, maps = _prep_all(x, edge_index, edge_attr, batch, depth, weights, G)
    nc = _build(plan, split=True)
    res = run_bass_kernel_spmd(nc, maps, list(range(NCORES)))
    return np.asarray(res.results[0]["out"]).reshape(G, 1).astype(np.float32)


# revision 12
# speedup vs baseline: 1.8283x; 1.8283x over previous
"""Trainium2 Bass kernel for nn_ChemModel (DMPNN-style message-passing GNN).

Self-contained: call kernel(**inputs) with the full (unsharded) inputs from
setup_inputs(); returns the full [N_GRAPHS, 1] float32 output.

Strategy (8 NeuronCores, SPMD — one program, per-core data):
  * Nodes/slots sharded by dst owner in contiguous ranges of N/8. Only h rows
    [0, N) evolve; rows >= N keep h0 and are recomputed during the final
    aggregation from host-packed [x[src]|edge_attr] columns (no gather).
  * Persistent transposed state hA_T [128h x SHP] in SBUF.
  * Message tables live in DRAM with a partition-major row order
    (row = core*SHP + (slot%128)*NB + slot//128) so the local shard is
    written with ONE contiguous DMA, then AllGathered (bf16).
  * Per-edge messages are fetched with a SINGLE dma_gather pass directly in
    dst-sorted order: edges are grouped by table-row % 4 and gathered with
    elem_step=4*H (int16 indices then cover the whole 100K-row table).
  * Scatter-add is one-hot matmul into sliding 256-slot PSUM windows with a
    host-computed, core-uniform window schedule (max over cores); padding
    edges carry dloc=-1 and self-neutralize in the one-hot.
  * Final phase: h-final rows for edge ids < N come from one more
    table+gather pass; edge ids >= N recompute h0 via matmul from host-packed
    inputs, overlapped with the collective + gathers.
"""
import math
import numpy as np
import ml_dtypes

import concourse.bass as bass
from concourse import bacc
import concourse.mybir as mybir
import concourse.tile as tile
from concourse.bass_utils import run_bass_kernel_spmd
from concourse import library_config

P = 128
H = 128
NCORES = 8
GIDX_N = 2048              # indices per dma_gather instruction
GCH = GIDX_N // P          # chunks per gather instruction (16)
ICOL = GIDX_N // 16        # idx columns per instruction in wrapped layout
WSL = 256                  # scatter window width in slots (2 blocks)
NRES = 4                   # residue groups (table row % 4)
JMAX = 8                   # max windows spanned by one chunk
F32 = mybir.dt.float32
BF16 = mybir.dt.bfloat16
I16 = mybir.dt.int16
BF16NP = ml_dtypes.bfloat16


def _relu():
    return mybir.ActivationFunctionType.Relu


def _wrap_idx16(flat):
    """[n] int array -> [128, n//16] int16 wrapped layout."""
    n = flat.shape[0]
    assert n % 16 == 0
    w = flat.reshape(n // 16, 16).T.astype(np.int16)
    return np.tile(w, (8, 1))


class _Plan:
    pass


class _Truncated(Exception):
    pass


def _build_stream_groups(rows_key, dloc, extra=None):
    """Split one core's edge stream into NRES residue groups sorted by dloc.

    Returns per group dict with idx ((row - r) // 4), dloc, and optionally a
    sorted copy of `extra` (per-edge payload columns [n, width])."""
    out = []
    for r in range(NRES):
        m = (rows_key % NRES) == r
        dl = dloc[m]
        rw = rows_key[m]
        o = np.argsort(dl, kind="stable")
        g = {"idx": (rw[o] - r) // NRES, "dloc": dl[o]}
        if extra is not None:
            g["extra"] = extra[m][o]
        out.append(g)
    return out


def _chunk_minmax(dloc_pad):
    """dloc_pad: [nch, P] with -1 pads -> per-chunk (min, max) over real
    entries; (inf, -inf) when empty."""
    real = dloc_pad >= 0
    mn = np.where(real, dloc_pad, np.inf).min(axis=1)
    mx = np.where(real, dloc_pad, -np.inf).max(axis=1)
    return mn, mx


def _host_prep(x, edge_index, edge_attr, batch, depth, G):
    N, E = x.shape[0], edge_index.shape[1]
    src = edge_index[0].astype(np.int64)
    dst = edge_index[1].astype(np.int64)
    batch = batch.astype(np.int64)
    x = np.asarray(x, np.float32)
    ea = np.asarray(edge_attr, np.float32)

    assert N % NCORES == 0
    NSH = N // NCORES
    NB = math.ceil(NSH / P)
    SHP = NB * P
    TBL = NCORES * SHP
    assert TBL % NRES == 0 and TBL // NRES <= 32512
    assert SHP % WSL == 0
    NW = SHP // WSL

    plan = _Plan()
    plan.N, plan.E, plan.G = N, E, G
    plan.NSH, plan.NB, plan.SHP, plan.TBL = NSH, NB, SHP, TBL
    plan.NW = NW
    plan.depth = int(depth)
    plan.GW = min(512, G)

    def row_of(n):
        s = n % NSH
        return (n // NSH) * SHP + (s % P) * NB + s // P

    row_src = row_of(src)

    # per-core edge partitions by dst owner
    core_of = dst // NSH
    per_core_loop = []   # all E edges (loop + used again conceptually)
    per_core_f0 = []     # edge ids < N  (gather h_final rows)
    per_core_f1 = []     # edge ids >= N (recompute h0)
    eids = np.arange(E)
    for k in range(NCORES):
        m = core_of == k
        ek = eids[m]
        dl = dst[m] - k * NSH
        per_core_loop.append(
            _build_stream_groups(row_src[ek], dl))
        m0 = ek < N
        per_core_f0.append(
            _build_stream_groups(row_of(ek[m0]), dl[m0]))
        ek1 = ek[m0 == False]  # noqa: E712
        xe = np.concatenate([x[src[ek1]], ea[ek1]], axis=1)  # [n1, 7]
        dl1 = dl[m0 == False]  # noqa: E712
        o = np.argsort(dl1, kind="stable")
        per_core_f1.append({"dloc": dl1[o], "extra": xe[o]})

    def finish_phase(groups_by_core, unit):
        """groups_by_core: [NCORES][NRES] dicts. unit: pad granularity
        (GIDX_N for gather streams). Produces uniform instruction counts,
        per-core idx/dloc tables and the shared window schedule."""
        ph = {}
        n_instr = []
        for g in range(NRES):
            mx = max(len(groups_by_core[k][g]["idx"]) for k in range(NCORES))
            n_instr.append(max(1, math.ceil(mx / unit)))
        ph["n_instr"] = n_instr
        nch_g = [ni * (unit // P) for ni in n_instr]
        ph["nch_g"] = nch_g
        # per-core padded dloc [group][core][nch, P]
        dl_pad = []
        for g in range(NRES):
            percore = []
            for k in range(NCORES):
                dl = groups_by_core[k][g]["dloc"]
                buf = np.full(nch_g[g] * P, -1.0, np.float64)
                buf[:len(dl)] = dl
                percore.append(buf.reshape(nch_g[g], P))
            dl_pad.append(percore)
        # window schedule (uniform across cores)
        sched = []
        for g in range(NRES):
            mns = np.full((NCORES, nch_g[g]), np.inf)
            mxs = np.full((NCORES, nch_g[g]), -np.inf)
            for k in range(NCORES):
                mns[k], mxs[k] = _chunk_minmax(dl_pad[g][k])
            mn = mns.min(axis=0)
            mx = mxs.max(axis=0)
            wfirst = np.where(np.isfinite(mn), mn // WSL, -1).astype(np.int64)
            wlast = np.where(np.isfinite(mx), mx // WSL, -2).astype(np.int64)
            span = (wlast - wfirst + 1).clip(min=0)
            assert span.max(initial=0) <= JMAX, f"window span {span.max()}"
            wl = {}
            for i in range(nch_g[g]):
                for w in range(wfirst[i], wlast[i] + 1):
                    wl.setdefault(w, []).append(i)
            # per chunk: list of (w, j, start, stop)
            chunks = []
            for i in range(nch_g[g]):
                lst = []
                for w in range(wfirst[i], wlast[i] + 1):
                    lst.append((int(w), int(w - wfirst[i]),
                                wl[w][0] == i, wl[w][-1] == i))
                chunks.append(lst)
            sched.append({"chunks": chunks, "wfirst": wfirst})
        ph["sched"] = sched
        # per-core relative dloc columns [128, sum(nch_g)] f32
        dloc_cols = []
        for k in range(NCORES):
            cols = []
            for g in range(NRES):
                wf = sched[g]["wfirst"]
                rel = dl_pad[g][k] - (wf[:, None] * WSL)
                rel[dl_pad[g][k] < 0] = -1.0
                cols.append(rel.reshape(nch_g[g], P).T)
            dloc_cols.append(
                np.ascontiguousarray(np.concatenate(cols, axis=1)
                                     .astype(np.float32)))
        ph["dloc"] = dloc_cols
        # per-core wrapped idx tables [128, sum(n_instr)*ICOL] i16
        idx_cols = []
        for k in range(NCORES):
            cols = []
            for g in range(NRES):
                idx = groups_by_core[k][g]["idx"]
                buf = np.zeros(n_instr[g] * unit, np.int64)  # pad -> row 0
                buf[:len(idx)] = idx
                for ii in range(n_instr[g]):
                    cols.append(_wrap_idx16(buf[ii * unit:(ii + 1) * unit]))
            idx_cols.append(np.ascontiguousarray(np.concatenate(cols, axis=1)))
        ph["ga"] = idx_cols
        return ph

    plan.loop = finish_phase(per_core_loop, GIDX_N)
    plan.f0 = finish_phase(per_core_f0, GIDX_N)

    # fin stream 1 (no gather): chunk/window schedule + packed [7, C1*P] cols
    C1 = max(1, max(math.ceil(len(per_core_f1[k]["dloc"]) / P)
                    for k in range(NCORES)))
    C1 = math.ceil(C1 / GCH) * GCH   # round to full 2048-col batches
    plan.C1 = C1
    dl_pad1, mns, mxs = [], np.full((NCORES, C1), np.inf), \
        np.full((NCORES, C1), -np.inf)
    for k in range(NCORES):
        dl = per_core_f1[k]["dloc"]
        buf = np.full(C1 * P, -1.0, np.float64)
        buf[:len(dl)] = dl
        dl_pad1.append(buf.reshape(C1, P))
        mns[k], mxs[k] = _chunk_minmax(dl_pad1[k])
    mn, mx = mns.min(axis=0), mxs.max(axis=0)
    wfirst = np.where(np.isfinite(mn), mn // WSL, -1).astype(np.int64)
    wlast = np.where(np.isfinite(mx), mx // WSL, -2).astype(np.int64)
    assert (wlast - wfirst + 1).clip(min=0).max(initial=0) <= JMAX
    wl = {}
    for i in range(C1):
        for w in range(wfirst[i], wlast[i] + 1):
            wl.setdefault(w, []).append(i)
    chunks1 = []
    for i in range(C1):
        lst = []
        for w in range(wfirst[i], wlast[i] + 1):
            lst.append((int(w), int(w - wfirst[i]),
                        wl[w][0] == i, wl[w][-1] == i))
        chunks1.append(lst)
    plan.f1_sched = chunks1
    plan.f1_dloc, plan.f1_xe = [], []
    for k in range(NCORES):
        rel = dl_pad1[k] - (wfirst[:, None] * WSL)
        rel[dl_pad1[k] < 0] = -1.0
        plan.f1_dloc.append(np.ascontiguousarray(
            rel.reshape(C1, P).T.astype(np.float32)))
        xe = np.zeros((C1 * P, 7), np.float32)
        n1 = len(per_core_f1[k]["dloc"])
        xe[:n1] = per_core_f1[k]["extra"]
        plan.f1_xe.append(np.ascontiguousarray(xe.T.astype(BF16NP)))

    # init: [7, SHP] = [x[src[slot]], ea[slot]] per core
    plan.xe_init = []
    for k in range(NCORES):
        sl = np.arange(k * NSH, (k + 1) * NSH)
        xe = np.zeros((SHP, 7), np.float32)
        xe[:NSH, :4] = x[src[sl]]
        xe[:NSH, 4:] = ea[sl]
        plan.xe_init.append(np.ascontiguousarray(xe.T.astype(BF16NP)))

    # node features for the final W_a matmul + pooling info
    plan.xT = []
    plan.batchloc = []
    plan.g_bases = []
    for k in range(NCORES):
        xs = np.zeros((SHP, 4), np.float32)
        xs[:NSH] = x[k * NSH:(k + 1) * NSH]
        plan.xT.append(np.ascontiguousarray(xs.T.astype(BF16NP)))
        gb = int(batch[k * NSH])
        ge = int(batch[(k + 1) * NSH - 1])
        assert ge - gb < plan.GW, f"graph span {ge - gb} >= {plan.GW}"
        plan.g_bases.append(gb)
        bl = np.full((SHP,), -1.0, np.float32)
        bl[:NSH] = batch[k * NSH:(k + 1) * NSH] - gb
        plan.batchloc.append(
            np.ascontiguousarray(bl.reshape(NB, P).T))

    plan.nA_L = sum(plan.loop["n_instr"])
    plan.nA_F = sum(plan.f0["n_instr"])
    plan.ncols_L = sum(plan.loop["nch_g"])
    plan.ncols_F = sum(plan.f0["nch_g"])
    return plan


# ----------------------------------------------------------------------------
# device kernel
# ----------------------------------------------------------------------------

def _build(plan, split=True, upto="full"):
    NB, SHP, TBL, NW = plan.NB, plan.SHP, plan.TBL, plan.NW
    G, GW = plan.G, plan.GW
    depth = plan.depth
    TBL4 = TBL // NRES

    nc = bacc.Bacc(num_devices=NCORES)

    def din(name, shape, dt=F32):
        return nc.declare_dram_parameter(name, list(shape), dt, isOutput=False)

    WmT = din("WmT", [H, H], BF16)
    Wi7T = din("Wi7T", [7, H], BF16)
    WaxT = din("WaxT", [4, H], BF16)
    WahT = din("WahT", [H, H], BF16)
    W1T = din("W1T", [H, 4 * H])
    W2T = din("W2T", [4 * H, H])
    WlastT = din("WlastT", [H, 1])
    b1r = din("b1r", [H, 4])
    b2r = din("b2r", [H, 1])
    blast = din("blast", [1, 1])
    iotaWJ = din("iotaWJ", [P, JMAX * WSL])
    iotaG = din("iotaG", [P, GW])
    ident = din("ident", [P, P])
    xeI_in = din("xeI", [7, SHP], BF16)
    xeF_in = din("xeF", [7, plan.C1 * P], BF16)
    xT_in = din("xT", [4, SHP], BF16)
    batchloc_in = din("batchloc", [P, NB])
    gaL_in = din("gaL", [P, plan.nA_L * ICOL], I16)
    gaF_in = din("gaF", [P, plan.nA_F * ICOL], I16)
    dlocL_in = din("dlocL", [P, plan.ncols_L])
    dlocF_in = din("dlocF", [P, plan.ncols_F])
    dlocF1_in = din("dlocF1", [P, plan.C1])

    out_ext = nc.declare_dram_parameter("out", [G, 1], F32, isOutput=True)

    RG = list(range(NCORES))

    with tile.TileContext(nc) as tc:
      try:
        nc.gpsimd.load_library(library_config.mlp)
        with (
            tc.tile_pool(name="cp", bufs=1) as cp,
            tc.tile_pool(name="sb", bufs=3) as sb,
            tc.tile_pool(name="ps", bufs=2, space="PSUM") as ps,
            tc.tile_pool(name="dr", bufs=1, space="DRAM") as dr,
        ):
            def cload(name, src_t):
                tl = cp.tile([src_t.shape[0], src_t.shape[1]], src_t.dtype,
                             name=name)
                nc.sync.dma_start(out=tl[:], in_=src_t[:, :])
                return tl

            WmT_s = cload("WmT_s", WmT)
            Wi7T_s = cload("Wi7T_s", Wi7T)
            WaxT_s = cload("WaxT_s", WaxT)
            WahT_s = cload("WahT_s", WahT)
            W1T_s = cload("W1T_s", W1T)
            W2T_f = []
            for f in range(4):
                tl = cp.tile([P, H], F32, name=f"W2T_{f}")
                nc.sync.dma_start(out=tl[:], in_=W2T[f * P:(f + 1) * P, :])
                W2T_f.append(tl)
            WlastT_s = cload("WlastT_s", WlastT)
            b1r_s = cload("b1r_s", b1r)
            b2r_s = cload("b2r_s", b2r)
            blast_s = cload("blast_s", blast)
            iotaWJ_s = cload("iotaWJ_s", iotaWJ)
            iotaG_s = cload("iotaG_s", iotaG)
            ident_s = cload("ident_s", ident)
            batchloc_s = cload("batchloc_s", batchloc_in)
            gaL_s = cload("gaL_s", gaL_in)
            gaF_s = cload("gaF_s", gaF_in)
            dlocL_s = cload("dlocL_s", dlocL_in)
            dlocF_s = cload("dlocF_s", dlocF_in)
            dlocF1_s = cload("dlocF1_s", dlocF1_in)

            hA_T = cp.tile([P, SHP], F32, name="hA_T")
            nacc_T = cp.tile([P, SHP], F32, name="nacc_T")
            nc.vector.memset(nacc_T[:], 0.0)
            tab_tile = cp.tile([P, NB, H], BF16, name="tab_tile")
            gfull = cp.tile([P, G], F32, name="gfull")

            loc = dr.tile([SHP, H], BF16, name="loc")
            tbls = [dr.tile([TBL, H], BF16, name=f"tbl{i}",
                            addr_space="Shared") for i in range(depth + 1)]
            gwin_local = dr.tile([P, GW], F32, name="gwin_local")
            gwin_all = dr.tile([NCORES * P, GW], F32, name="gwin_all",
                               addr_space="Shared")

            def allgather(local, table):
                nc.gpsimd.collective_compute(
                    "AllGather", mybir.AluOpType.bypass,
                    replica_groups=[RG], ins=[local[:]], outs=[table[:]])

            def gather(dst_ap, tbl_t, res, idx_sb, inst_col):
                base = tbl_t[:]
                in_ap = bass.AP(base.tensor, base.offset + res * H,
                                [[NRES * H, TBL4], [1, H]])
                nc.gpsimd.dma_gather(
                    out_ap=dst_ap, in_ap=in_ap,
                    idxs_ap=idx_sb[:, inst_col * ICOL:(inst_col + 1) * ICOL],
                    num_idxs=GIDX_N, num_idxs_reg=GIDX_N,
                    elem_size=H, elem_step=NRES * H, single_packet=False)

            # ------------- scatter machinery (shared schedule walker) -------
            def scatter_chunks(chunk_iter, acc):
                """chunk_iter yields (msg_tile, cslice, dloc_col, wlist).
                One-hot matmul per (chunk, window) into a fresh PSUM tile,
                immediately added into `acc` [P, SHP]."""
                for msg, csl, dcol, wlist in chunk_iter:
                    for (w, j, st, sp) in wlist:
                        oh = sb.tile([P, WSL], BF16, name="oh", tag="oh",
                                     bufs=4)
                        nc.vector.tensor_tensor(
                            out=oh[:],
                            in0=dcol.to_broadcast([P, WSL]),
                            in1=iotaWJ_s[:, j * WSL:(j + 1) * WSL],
                            op=mybir.AluOpType.is_equal)
                        pw = ps.tile([P, WSL], F32, name="pw", tag="pacc",
                                     space="PSUM", bufs=3)
                        nc.tensor.matmul(out=pw[:], lhsT=msg[csl],
                                         rhs=oh[:], start=True, stop=True)
                        a = acc[:, w * WSL:(w + 1) * WSL]
                        nc.vector.tensor_add(out=a, in0=a, in1=pw[:])

            def gather_phase_chunks(ph, dloc_s, ga_s, tbl_t):
                """Generator of scatter_chunks items for a gather phase.
                Gathers are emitted with lookahead 2 so that tile-pool buffer
                reuse (bufs=3) never outruns already-emitted consumers."""
                LA = 2
                icol = 0
                col0 = 0
                for g in range(NRES):
                    sched = ph["sched"][g]["chunks"]
                    nin = ph["n_instr"][g]
                    tiles = [None] * nin

                    def emit(ii, icol0=icol):
                        gt = sb.tile([P, GCH, H], BF16, name="gt", tag="gat")
                        gather(gt[:], tbl_t, g, ga_s, icol0 + ii)
                        tiles[ii] = gt
                    for ii in range(min(LA, nin)):
                        emit(ii)
                    for ii in range(nin):
                        if ii + LA < nin:
                            emit(ii + LA)
                        for c in range(GCH):
                            i = ii * GCH + c
                            wlist = sched[i]
                            if not wlist:
                                continue
                            yield (tiles[ii], np.s_[:, c, :],
                                   dloc_s[:, col0 + i:col0 + i + 1], wlist)
                    icol += nin
                    col0 += ph["nch_g"][g]

            # ---------------- init ----------------
            for bi in range(math.ceil(NB / GCH)):
                b0 = bi * GCH
                b1_ = min(NB, b0 + GCH)
                xei = sb.tile([7, GIDX_N], BF16, name="xei", tag="xet",
                              bufs=2)
                nc.sync.dma_start(
                    out=xei[:, :(b1_ - b0) * P],
                    in_=xeI_in[:, b0 * P:b1_ * P])
                for b in range(b0, b1_):
                    pi = ps.tile([P, H], F32, name="pi", tag="ptmp",
                                 space="PSUM")
                    nc.tensor.matmul(out=pi[:], lhsT=Wi7T_s[:],
                                     rhs=xei[:, (b - b0) * P:(b - b0 + 1) * P],
                                     start=True, stop=True)
                    nc.scalar.activation(out=hA_T[:, b * P:(b + 1) * P],
                                         in_=pi[:], func=_relu())

            # ---------------- message-passing iterations ----------------
            nloops = {"init": 0, "loop1": 1, "noscat": 1}.get(upto, depth)
            for it in range(nloops):
                # mA = relu(W_m h) -> bf16 table tile -> DRAM -> AllGather
                for b in range(NB):
                    hb = sb.tile([P, P], BF16, name="hb", tag="hb")
                    nc.vector.tensor_copy(out=hb[:],
                                          in_=hA_T[:, b * P:(b + 1) * P])
                    pm = ps.tile([P, H], F32, name="pm", tag="ptmp",
                                 space="PSUM")
                    nc.tensor.matmul(out=pm[:], lhsT=hb[:], rhs=WmT_s[:],
                                     start=True, stop=True)
                    nc.scalar.activation(out=tab_tile[:, b, :], in_=pm[:],
                                         func=_relu())
                nc.sync.dma_start(
                    out=loc[:, :].rearrange("(p b) h -> p (b h)", p=P),
                    in_=tab_tile[:])
                allgather(loc, tbls[it])
                if upto == "noscat":
                    for gi2 in gather_phase_chunks(plan.loop, dlocL_s, gaL_s,
                                                   tbls[it]):
                        pass
                else:
                    scatter_chunks(
                        gather_phase_chunks(plan.loop, dlocL_s, gaL_s,
                                            tbls[it]),
                        hA_T)

            # ---------------- final aggregation ----------------
            if upto in ("init", "loop1", "noscat", "loop"):
                stub = sb.tile([1, 1], F32, name="stub", tag="oc", bufs=2)
                nc.vector.tensor_copy(out=stub[:], in_=hA_T[0:1, 0:1])
                nc.sync.dma_start(out=out_ext[0:1, :], in_=stub[:])
                raise _Truncated()
            # h_final table (transposed state) -> DRAM -> AllGather
            for b in range(NB):
                pt = ps.tile([P, H], F32, name="pt", tag="ptmp", space="PSUM")
                nc.tensor.transpose(out=pt[:], in_=hA_T[:, b * P:(b + 1) * P],
                                    identity=ident_s[:])
                nc.vector.tensor_copy(out=tab_tile[:, b, :], in_=pt[:])
            nc.sync.dma_start(
                out=loc[:, :].rearrange("(p b) h -> p (b h)", p=P),
                in_=tab_tile[:])
            allgather(loc, tbls[depth])

            # stream 1: h0 recompute for edge ids >= N (overlaps AG + gathers)
            def f1_chunks():
                nbatch = plan.C1 // GCH
                for bi in range(nbatch):
                    xet = sb.tile([7, GIDX_N], BF16, name="xet", tag="xet",
                                  bufs=2)
                    nc.sync.dma_start(
                        out=xet[:],
                        in_=xeF_in[:, bi * GIDX_N:(bi + 1) * GIDX_N])
                    for c in range(GCH):
                        i = bi * GCH + c
                        wlist = plan.f1_sched[i]
                        if not wlist:
                            continue
                        ph0 = ps.tile([P, H], F32, name="ph0", tag="ptmp",
                                      space="PSUM")
                        nc.tensor.matmul(out=ph0[:],
                                         lhsT=xet[:, c * P:(c + 1) * P],
                                         rhs=Wi7T_s[:], start=True, stop=True)
                        msg = sb.tile([P, H], BF16, name="msg", tag="msg",
                                      bufs=4)
                        nc.scalar.activation(out=msg[:], in_=ph0[:],
                                             func=_relu())
                        yield (msg, np.s_[:, :],
                               dlocF1_s[:, i:i + 1], wlist)

            # emit: f0 gathers first (Pool/DMA), then f1 (PE) runs under them,
            # then f0 scatter consumes the gathered tiles.
            f0_iter = gather_phase_chunks(plan.f0, dlocF_s, gaF_s, tbls[depth])
            scatter_chunks(f1_chunks(), nacc_T)
            scatter_chunks(f0_iter, nacc_T)

            # ---------------- node_emb + pooling ----------------
            gps = ps.tile([P, GW], F32, name="gps", tag="gps", space="PSUM",
                          bufs=1)
            for b in range(NB):
                if b % GCH == 0:
                    b0 = b
                    b1_ = min(NB, b0 + GCH)
                    xtt = sb.tile([4, GIDX_N], BF16, name="xtt", tag="xet",
                                  bufs=2)
                    nc.sync.dma_start(
                        out=xtt[:, :(b1_ - b0) * P],
                        in_=xT_in[:, b0 * P:b1_ * P])
                nb16 = sb.tile([P, P], BF16, name="nb16", tag="hb")
                nc.vector.tensor_copy(out=nb16[:],
                                      in_=nacc_T[:, b * P:(b + 1) * P])
                p2 = ps.tile([P, H], F32, name="p2", tag="ptmp", space="PSUM")
                nc.tensor.matmul(out=p2[:], lhsT=nb16[:],
                                 rhs=WahT_s[:], start=True, stop=False)
                nc.tensor.matmul(out=p2[:],
                                 lhsT=xtt[:, (b - b0) * P:(b - b0 + 1) * P],
                                 rhs=WaxT_s[:], start=False, stop=True)
                ne2 = sb.tile([P, H], BF16, name="ne2", tag="msg", bufs=4)
                nc.scalar.activation(out=ne2[:], in_=p2[:], func=_relu())
                ohg = sb.tile([P, GW], BF16, name="ohg", tag="ohg")
                nc.vector.tensor_tensor(
                    out=ohg[:],
                    in0=batchloc_s[:, b:b + 1].to_broadcast([P, GW]),
                    in1=iotaG_s[:], op=mybir.AluOpType.is_equal)
                nc.tensor.matmul(out=gps[:], lhsT=ne2[:], rhs=ohg[:],
                                 start=(b == 0), stop=(b == NB - 1))

            tgw = sb.tile([P, GW], F32, name="tgw", tag="ohg")
            nc.vector.tensor_copy(out=tgw[:], in_=gps[:])
            nc.sync.dma_start(out=gwin_local[:, :], in_=tgw[:])
            allgather(gwin_local, gwin_all)
            nc.vector.memset(gfull[:], 0.0)
            for j in range(NCORES):
                wj = min(GW, G - plan.g_bases[j])
                tw = sb.tile([P, GW], F32, name="twj", tag="ohg")
                nc.sync.dma_start(out=tw[:], in_=gwin_all[j * P:(j + 1) * P, :])
                nc.vector.tensor_add(
                    out=gfull[:, plan.g_bases[j]:plan.g_bases[j] + wj],
                    in0=gfull[:, plan.g_bases[j]:plan.g_bases[j] + wj],
                    in1=tw[:, :wj])

            # ---------------- FFN (replicated on all cores) ----------------
            NGC = math.ceil(G / 512)
            for gc in range(NGC):
                g0, g1 = gc * 512, min((gc + 1) * 512, G)
                pz2 = ps.tile([P, 512], F32, name="pz2", tag="pz2",
                              space="PSUM", bufs=1)
                for f in range(4):
                    pz = ps.tile([P, 512], F32, name="pz", tag="ptmp",
                                 space="PSUM")
                    nc.tensor.matmul(out=pz[:, :g1 - g0],
                                     lhsT=W1T_s[:, f * P:(f + 1) * P],
                                     rhs=gfull[:, g0:g1], start=True, stop=True)
                    z1 = sb.tile([P, 512], F32, name="z1", tag="z1", bufs=2)
                    nc.scalar.activation(out=z1[:, :g1 - g0],
                                         in_=pz[:, :g1 - g0], func=_relu(),
                                         bias=b1r_s[:, f:f + 1])
                    nc.tensor.matmul(out=pz2[:, :g1 - g0], lhsT=W2T_f[f][:],
                                     rhs=z1[:, :g1 - g0], start=(f == 0),
                                     stop=(f == 3))
                z2 = sb.tile([P, 512], F32, name="z2", tag="z1", bufs=2)
                nc.vector.tensor_add(
                    out=z2[:, :g1 - g0], in0=pz2[:, :g1 - g0],
                    in1=b2r_s[:, 0:1].to_broadcast([P, g1 - g0]))
                po = ps.tile([1, 512], F32, name="po", tag="ptmp",
                             space="PSUM")
                nc.tensor.matmul(out=po[:, :g1 - g0], lhsT=WlastT_s[:],
                                 rhs=z2[:, :g1 - g0], start=True, stop=True)
                oc = sb.tile([1, 512], F32, name="oc", tag="oc", bufs=2)
                nc.vector.tensor_add(
                    out=oc[:, :g1 - g0], in0=po[:, :g1 - g0],
                    in1=blast_s[0:1, 0:1].to_broadcast([1, g1 - g0]))
                nc.sync.dma_start(out=out_ext[g0:g1, :], in_=oc[:, :g1 - g0])

      except _Truncated:
        pass
    nc.compile()
    if split:
        _split_excess_waits(nc)
    return nc


def _split_excess_waits(nc, max_waits=1):
    k = 0
    for f in nc.m.functions:
        for bb in f.blocks:
            new = []
            for ins in bb.instructions:
                si = ins.sync_info
                if si is not None and len(si.on_wait) > max_waits:
                    waits = list(si.on_wait)
                    for w in waits[:-max_waits]:
                        nop = mybir.InstNoOp(name=f"I-waitsplit-{k}",
                                             engine=ins.engine)
                        k += 1
                        nop.sync_info = mybir.SyncInfo(on_wait=[w],
                                                       on_update=[])
                        new.append(nop)
                    si.on_wait = waits[-max_waits:]
                new.append(ins)
            bb.instructions = new
    return k


# ----------------------------------------------------------------------------
# inputs
# ----------------------------------------------------------------------------

def _in_maps(plan, weights):
    com = {
        "WmT": np.ascontiguousarray(weights["W_m"].T).astype(BF16NP),
        "Wi7T": np.ascontiguousarray(weights["W_i"].T).astype(BF16NP),
        "WaxT": np.ascontiguousarray(weights["W_a"][:, :4].T).astype(BF16NP),
        "WahT": np.ascontiguousarray(weights["W_a"][:, 4:].T).astype(BF16NP),
        "W1T": np.ascontiguousarray(weights["W1"].T),
        "W2T": np.ascontiguousarray(weights["W2"].T),
        "WlastT": np.ascontiguousarray(weights["W_last"].T),
        "b1r": np.ascontiguousarray(weights["b1"].reshape(4, H).T),
        "b2r": weights["b2"].reshape(H, 1).copy(),
        "blast": weights["b_last"].reshape(1, 1).copy(),
        "iotaWJ": np.tile(np.arange(JMAX * WSL, dtype=np.float32), (P, 1)),
        "iotaG": np.tile(np.arange(plan.GW, dtype=np.float32), (P, 1)),
        "ident": np.eye(P, dtype=np.float32),
    }
    maps = []
    for k in range(NCORES):
        m = dict(com)
        m["xeI"] = plan.xe_init[k]
        m["xeF"] = plan.f1_xe[k]
        m["xT"] = plan.xT[k]
        m["batchloc"] = plan.batchloc[k]
        m["gaL"] = plan.loop["ga"][k]
        m["gaF"] = plan.f0["ga"][k]
        m["dlocL"] = plan.loop["dloc"][k]
        m["dlocF"] = plan.f0["dloc"][k]
        m["dlocF1"] = plan.f1_dloc[k]
        maps.append(m)
    return maps


def _prep_all(x, edge_index, edge_attr, batch, depth, weights, G):
    plan = _host_prep(np.asarray(x, np.float32), np.asarray(edge_index),
                      np.asarray(edge_attr, np.float32), np.asarray(batch),
                      int(depth), G)
    maps = _in_maps(plan, weights)
    return plan, maps


def kernel(x, edge_index, edge_attr, batch, depth,
           W_i, W_m, W_a, W1, b1, W2, b2, W_last, b_last):
    weights = {
        "W_i": np.asarray(W_i, np.float32), "W_m": np.asarray(W_m, np.float32),
        "W_a": np.asarray(W_a, np.float32), "W1": np.asarray(W1, np.float32),
        "b1": np.asarray(b1, np.float32), "W2": np.asarray(W2, np.float32),
        "b2": np.asarray(b2, np.float32),
        "W_last": np.asarray(W_last, np.float32),
        "b_last": np.asarray(b_last, np.float32),
    }
    G = 2048
    plan, maps = _prep_all(x, edge_index, edge_attr, batch, depth, weights, G)
    nc = _build(plan, split=True)
    res = run_bass_kernel_spmd(nc, maps, list(range(NCORES)))
    return np.asarray(res.results[0]["out"]).reshape(G, 1).astype(np.float32)


# revision 13
# speedup vs baseline: 2.4511x; 1.3407x over previous
"""Trainium2 Bass kernel for nn_ChemModel (DMPNN-style message-passing GNN).

Self-contained: call kernel(**inputs) with the full (unsharded) inputs from
setup_inputs(); returns the full [N_GRAPHS, 1] float32 output.

Strategy (8 NeuronCores, SPMD — one program, per-core data):
  * Nodes/slots sharded by dst owner in contiguous ranges of N/8. Only h rows
    [0, N) evolve; rows >= N keep h0 and are recomputed during the final
    aggregation from host-packed [x[src]|edge_attr] columns (no gather).
  * Persistent transposed state hA_T [128h x SHP] in SBUF.
  * Message tables live in DRAM with a partition-major row order
    (row = core*SHP + (slot%128)*NB + slot//128) so the local shard is
    written with ONE contiguous DMA, then AllGathered (bf16).
  * Per-edge messages are fetched with a SINGLE dma_gather pass directly in
    dst-sorted order: edges are grouped by table-row % 4 and gathered with
    elem_step=4*H (int16 indices then cover the whole 100K-row table).
  * Scatter-add is one-hot matmul into sliding 256-slot PSUM windows with a
    host-computed, core-uniform window schedule (max over cores); padding
    edges carry dloc=-1 and self-neutralize in the one-hot.
  * Final phase: h-final rows for edge ids < N come from one more
    table+gather pass; edge ids >= N recompute h0 via matmul from host-packed
    inputs, overlapped with the collective + gathers.
"""
import math
import numpy as np
import ml_dtypes

import concourse.bass as bass
from concourse import bacc
import concourse.mybir as mybir
import concourse.tile as tile
from concourse.bass_utils import run_bass_kernel_spmd
from concourse import library_config

P = 128
H = 128
NCORES = 8
GIDX_N = 2048              # indices per dma_gather instruction
GCH = GIDX_N // P          # chunks per gather instruction (16)
ICOL = GIDX_N // 16        # idx columns per instruction in wrapped layout
WSL = 256                  # scatter window width in slots (2 blocks)
NRES = 4                   # residue groups (table row % 4)
JMAX = 8                   # max windows spanned by one chunk
F32 = mybir.dt.float32
BF16 = mybir.dt.bfloat16
I16 = mybir.dt.int16
BF16NP = ml_dtypes.bfloat16


def _relu():
    return mybir.ActivationFunctionType.Relu


def _wrap_idx16(flat):
    """[n] int array -> [128, n//16] int16 wrapped layout."""
    n = flat.shape[0]
    assert n % 16 == 0
    w = flat.reshape(n // 16, 16).T.astype(np.int16)
    return np.tile(w, (8, 1))


class _Plan:
    pass


class _Truncated(Exception):
    pass


def _build_stream_groups(rows_key, dloc, extra=None):
    """Split one core's edge stream into NRES residue groups sorted by dloc.

    Returns per group dict with idx ((row - r) // 4), dloc, and optionally a
    sorted copy of `extra` (per-edge payload columns [n, width])."""
    out = []
    for r in range(NRES):
        m = (rows_key % NRES) == r
        dl = dloc[m]
        rw = rows_key[m]
        o = np.argsort(dl, kind="stable")
        g = {"idx": (rw[o] - r) // NRES, "dloc": dl[o]}
        if extra is not None:
            g["extra"] = extra[m][o]
        out.append(g)
    return out


def _chunk_minmax(dloc_pad):
    """dloc_pad: [nch, P] with -1 pads -> per-chunk (min, max) over real
    entries; (inf, -inf) when empty."""
    real = dloc_pad >= 0
    mn = np.where(real, dloc_pad, np.inf).min(axis=1)
    mx = np.where(real, dloc_pad, -np.inf).max(axis=1)
    return mn, mx


def _host_prep(x, edge_index, edge_attr, batch, depth, G):
    N, E = x.shape[0], edge_index.shape[1]
    src = edge_index[0].astype(np.int64)
    dst = edge_index[1].astype(np.int64)
    batch = batch.astype(np.int64)
    x = np.asarray(x, np.float32)
    ea = np.asarray(edge_attr, np.float32)

    assert N % NCORES == 0
    NSH = N // NCORES
    NB = math.ceil(NSH / P)
    SHP = NB * P
    TBL = NCORES * SHP
    assert TBL % NRES == 0 and TBL // NRES <= 32512
    assert SHP % WSL == 0
    NW = SHP // WSL

    plan = _Plan()
    plan.N, plan.E, plan.G = N, E, G
    plan.NSH, plan.NB, plan.SHP, plan.TBL = NSH, NB, SHP, TBL
    plan.NW = NW
    plan.depth = int(depth)
    plan.GW = min(512, G)

    def row_of(n):
        s = n % NSH
        return (n // NSH) * SHP + (s % P) * NB + s // P

    row_src = row_of(src)

    # per-core edge partitions by dst owner
    core_of = dst // NSH
    per_core_loop = []   # all E edges (loop + used again conceptually)
    per_core_f0 = []     # edge ids < N  (gather h_final rows)
    per_core_f1 = []     # edge ids >= N (recompute h0)
    eids = np.arange(E)
    for k in range(NCORES):
        m = core_of == k
        ek = eids[m]
        dl = dst[m] - k * NSH
        per_core_loop.append(
            _build_stream_groups(row_src[ek], dl))
        m0 = ek < N
        per_core_f0.append(
            _build_stream_groups(row_of(ek[m0]), dl[m0]))
        ek1 = ek[m0 == False]  # noqa: E712
        xe = np.concatenate([x[src[ek1]], ea[ek1]], axis=1)  # [n1, 7]
        dl1 = dl[m0 == False]  # noqa: E712
        o = np.argsort(dl1, kind="stable")
        per_core_f1.append({"dloc": dl1[o], "extra": xe[o]})

    def finish_phase(groups_by_core, unit):
        """groups_by_core: [NCORES][NRES] dicts. unit: pad granularity
        (GIDX_N for gather streams). Produces uniform instruction counts,
        per-core idx/dloc tables and the shared window schedule."""
        ph = {}
        n_instr = []
        for g in range(NRES):
            mx = max(len(groups_by_core[k][g]["idx"]) for k in range(NCORES))
            n_instr.append(max(1, math.ceil(mx / unit)))
        ph["n_instr"] = n_instr
        nch_g = [ni * (unit // P) for ni in n_instr]
        ph["nch_g"] = nch_g
        # per-core padded dloc [group][core][nch, P]
        dl_pad = []
        for g in range(NRES):
            percore = []
            for k in range(NCORES):
                dl = groups_by_core[k][g]["dloc"]
                buf = np.full(nch_g[g] * P, -1.0, np.float64)
                buf[:len(dl)] = dl
                percore.append(buf.reshape(nch_g[g], P))
            dl_pad.append(percore)
        # window schedule (uniform across cores)
        sched = []
        for g in range(NRES):
            mns = np.full((NCORES, nch_g[g]), np.inf)
            mxs = np.full((NCORES, nch_g[g]), -np.inf)
            for k in range(NCORES):
                mns[k], mxs[k] = _chunk_minmax(dl_pad[g][k])
            mn = mns.min(axis=0)
            mx = mxs.max(axis=0)
            wfirst = np.where(np.isfinite(mn), mn // WSL, -1).astype(np.int64)
            wlast = np.where(np.isfinite(mx), mx // WSL, -2).astype(np.int64)
            span = (wlast - wfirst + 1).clip(min=0)
            assert span.max(initial=0) <= JMAX, f"window span {span.max()}"
            wl = {}
            for i in range(nch_g[g]):
                for w in range(wfirst[i], wlast[i] + 1):
                    wl.setdefault(w, []).append(i)
            # per chunk: list of (w, j, start, stop)
            chunks = []
            for i in range(nch_g[g]):
                lst = []
                for w in range(wfirst[i], wlast[i] + 1):
                    lst.append((int(w), int(w - wfirst[i]),
                                wl[w][0] == i, wl[w][-1] == i))
                chunks.append(lst)
            sched.append({"chunks": chunks, "wfirst": wfirst})
        ph["sched"] = sched
        # per-core relative dloc columns [128, sum(nch_g)] f32
        dloc_cols = []
        for k in range(NCORES):
            cols = []
            for g in range(NRES):
                wf = sched[g]["wfirst"]
                rel = dl_pad[g][k] - (wf[:, None] * WSL)
                rel[dl_pad[g][k] < 0] = -1.0
                cols.append(rel.reshape(nch_g[g], P).T)
            dloc_cols.append(
                np.ascontiguousarray(np.concatenate(cols, axis=1)
                                     .astype(np.float32)))
        ph["dloc"] = dloc_cols
        # per-core wrapped idx tables [128, sum(n_instr)*ICOL] i16
        idx_cols = []
        for k in range(NCORES):
            cols = []
            for g in range(NRES):
                idx = groups_by_core[k][g]["idx"]
                buf = np.zeros(n_instr[g] * unit, np.int64)  # pad -> row 0
                buf[:len(idx)] = idx
                for ii in range(n_instr[g]):
                    cols.append(_wrap_idx16(buf[ii * unit:(ii + 1) * unit]))
            idx_cols.append(np.ascontiguousarray(np.concatenate(cols, axis=1)))
        ph["ga"] = idx_cols
        return ph

    plan.loop = finish_phase(per_core_loop, GIDX_N)
    plan.f0 = finish_phase(per_core_f0, GIDX_N)

    # fin stream 1 (no gather): chunk/window schedule + packed [7, C1*P] cols
    C1 = max(1, max(math.ceil(len(per_core_f1[k]["dloc"]) / P)
                    for k in range(NCORES)))
    C1 = math.ceil(C1 / GCH) * GCH   # round to full 2048-col batches
    plan.C1 = C1
    dl_pad1, mns, mxs = [], np.full((NCORES, C1), np.inf), \
        np.full((NCORES, C1), -np.inf)
    for k in range(NCORES):
        dl = per_core_f1[k]["dloc"]
        buf = np.full(C1 * P, -1.0, np.float64)
        buf[:len(dl)] = dl
        dl_pad1.append(buf.reshape(C1, P))
        mns[k], mxs[k] = _chunk_minmax(dl_pad1[k])
    mn, mx = mns.min(axis=0), mxs.max(axis=0)
    wfirst = np.where(np.isfinite(mn), mn // WSL, -1).astype(np.int64)
    wlast = np.where(np.isfinite(mx), mx // WSL, -2).astype(np.int64)
    assert (wlast - wfirst + 1).clip(min=0).max(initial=0) <= JMAX
    wl = {}
    for i in range(C1):
        for w in range(wfirst[i], wlast[i] + 1):
            wl.setdefault(w, []).append(i)
    chunks1 = []
    for i in range(C1):
        lst = []
        for w in range(wfirst[i], wlast[i] + 1):
            lst.append((int(w), int(w - wfirst[i]),
                        wl[w][0] == i, wl[w][-1] == i))
        chunks1.append(lst)
    plan.f1_sched = chunks1
    plan.f1_dloc, plan.f1_xe = [], []
    for k in range(NCORES):
        rel = dl_pad1[k] - (wfirst[:, None] * WSL)
        rel[dl_pad1[k] < 0] = -1.0
        plan.f1_dloc.append(np.ascontiguousarray(
            rel.reshape(C1, P).T.astype(np.float32)))
        xe = np.zeros((C1 * P, 7), np.float32)
        n1 = len(per_core_f1[k]["dloc"])
        xe[:n1] = per_core_f1[k]["extra"]
        plan.f1_xe.append(np.ascontiguousarray(xe.T.astype(BF16NP)))

    # init: [7, SHP] = [x[src[slot]], ea[slot]] per core
    plan.xe_init = []
    for k in range(NCORES):
        sl = np.arange(k * NSH, (k + 1) * NSH)
        xe = np.zeros((SHP, 7), np.float32)
        xe[:NSH, :4] = x[src[sl]]
        xe[:NSH, 4:] = ea[sl]
        plan.xe_init.append(np.ascontiguousarray(xe.T.astype(BF16NP)))

    # node features for the final W_a matmul + pooling info
    plan.xT = []
    plan.batchloc = []
    plan.g_bases = []
    for k in range(NCORES):
        xs = np.zeros((SHP, 4), np.float32)
        xs[:NSH] = x[k * NSH:(k + 1) * NSH]
        plan.xT.append(np.ascontiguousarray(xs.T.astype(BF16NP)))
        gb = int(batch[k * NSH])
        ge = int(batch[(k + 1) * NSH - 1])
        assert ge - gb < plan.GW, f"graph span {ge - gb} >= {plan.GW}"
        plan.g_bases.append(gb)
        bl = np.full((SHP,), -1.0, np.float32)
        bl[:NSH] = batch[k * NSH:(k + 1) * NSH] - gb
        plan.batchloc.append(
            np.ascontiguousarray(bl.reshape(NB, P).T))

    plan.nA_L = sum(plan.loop["n_instr"])
    plan.nA_F = sum(plan.f0["n_instr"])
    plan.ncols_L = sum(plan.loop["nch_g"])
    plan.ncols_F = sum(plan.f0["nch_g"])
    return plan


# ----------------------------------------------------------------------------
# device kernel
# ----------------------------------------------------------------------------

def _build(plan, split=True, upto="full"):
    NB, SHP, TBL, NW = plan.NB, plan.SHP, plan.TBL, plan.NW
    G, GW = plan.G, plan.GW
    depth = plan.depth
    TBL4 = TBL // NRES

    nc = bacc.Bacc(num_devices=NCORES)

    def din(name, shape, dt=F32):
        return nc.declare_dram_parameter(name, list(shape), dt, isOutput=False)

    WmT = din("WmT", [H, H], BF16)
    Wi7T = din("Wi7T", [7, H], BF16)
    WaxT = din("WaxT", [4, H], BF16)
    WahT = din("WahT", [H, H], BF16)
    W1T = din("W1T", [H, 4 * H])
    W2T = din("W2T", [4 * H, H])
    WlastT = din("WlastT", [H, 1])
    b1r = din("b1r", [H, 4])
    b2r = din("b2r", [H, 1])
    blast = din("blast", [1, 1])
    iotaWJ = din("iotaWJ", [P, JMAX * WSL])
    iotaG = din("iotaG", [P, GW])
    ident = din("ident", [P, P])
    xeI_in = din("xeI", [7, SHP], BF16)
    xeF_in = din("xeF", [7, plan.C1 * P], BF16)
    xT_in = din("xT", [4, SHP], BF16)
    batchloc_in = din("batchloc", [P, NB])
    gaL_in = din("gaL", [P, plan.nA_L * ICOL], I16)
    gaF_in = din("gaF", [P, plan.nA_F * ICOL], I16)
    dlocL_in = din("dlocL", [P, plan.ncols_L])
    dlocF_in = din("dlocF", [P, plan.ncols_F])
    dlocF1_in = din("dlocF1", [P, plan.C1])

    out_ext = nc.declare_dram_parameter("out", [G, 1], F32, isOutput=True)

    RG = list(range(NCORES))

    with tile.TileContext(nc) as tc:
      try:
        nc.gpsimd.load_library(library_config.mlp)
        with (
            tc.tile_pool(name="cp", bufs=1) as cp,
            tc.tile_pool(name="sb", bufs=3) as sb,
            tc.tile_pool(name="ps", bufs=2, space="PSUM") as ps,
            tc.tile_pool(name="dr", bufs=1, space="DRAM") as dr,
        ):
            def cload(name, src_t):
                tl = cp.tile([src_t.shape[0], src_t.shape[1]], src_t.dtype,
                             name=name)
                nc.sync.dma_start(out=tl[:], in_=src_t[:, :])
                return tl

            WmT_s = cload("WmT_s", WmT)
            Wi7T_s = cload("Wi7T_s", Wi7T)
            WaxT_s = cload("WaxT_s", WaxT)
            WahT_s = cload("WahT_s", WahT)
            W1T_s = cload("W1T_s", W1T)
            W2T_f = []
            for f in range(4):
                tl = cp.tile([P, H], F32, name=f"W2T_{f}")
                nc.sync.dma_start(out=tl[:], in_=W2T[f * P:(f + 1) * P, :])
                W2T_f.append(tl)
            WlastT_s = cload("WlastT_s", WlastT)
            b1r_s = cload("b1r_s", b1r)
            b2r_s = cload("b2r_s", b2r)
            blast_s = cload("blast_s", blast)
            iotaWJ_s = cload("iotaWJ_s", iotaWJ)
            iotaG_s = cload("iotaG_s", iotaG)
            ident_s = cload("ident_s", ident)
            batchloc_s = cload("batchloc_s", batchloc_in)
            gaL_s = cload("gaL_s", gaL_in)
            gaF_s = cload("gaF_s", gaF_in)
            dlocL_s = cload("dlocL_s", dlocL_in)
            dlocF_s = cload("dlocF_s", dlocF_in)
            dlocF1_s = cload("dlocF1_s", dlocF1_in)

            hA_T = cp.tile([P, SHP], F32, name="hA_T")
            nacc_T = cp.tile([P, SHP], F32, name="nacc_T")
            nc.vector.memset(nacc_T[:], 0.0)
            tab_tile = cp.tile([P, NB, H], BF16, name="tab_tile")
            gfull = cp.tile([P, G], F32, name="gfull")

            loc = dr.tile([SHP, H], BF16, name="loc")
            tbls = [dr.tile([TBL, H], BF16, name=f"tbl{i}")
                    for i in range(depth + 1)]
            gwin_local = dr.tile([P, GW], F32, name="gwin_local")
            gwin_all = dr.tile([NCORES * P, GW], F32, name="gwin_all",
                               addr_space="Shared")

            def allgather(local, table):
                nc.gpsimd.collective_compute(
                    "AllGather", mybir.AluOpType.bypass,
                    replica_groups=[RG], ins=[local[:]], outs=[table[:]])

            def gather(dst_ap, tbl_t, res, idx_sb, inst_col):
                base = tbl_t[:]
                in_ap = bass.AP(base.tensor, base.offset + res * H,
                                [[NRES * H, TBL4], [1, H]])
                nc.gpsimd.dma_gather(
                    out_ap=dst_ap, in_ap=in_ap,
                    idxs_ap=idx_sb[:, inst_col * ICOL:(inst_col + 1) * ICOL],
                    num_idxs=GIDX_N, num_idxs_reg=GIDX_N,
                    elem_size=H, elem_step=NRES * H, single_packet=False)

            # ------------- scatter machinery (shared schedule walker) -------
            def scatter_chunks(chunk_iter, acc):
                """chunk_iter yields (msg_tile, cslice, dloc_col, wlist).
                One-hot matmul per (chunk, window) into a fresh PSUM tile,
                immediately added into `acc` [P, SHP]."""
                for msg, csl, dcol, wlist in chunk_iter:
                    for (w, j, st, sp) in wlist:
                        oh = sb.tile([P, WSL], BF16, name="oh", tag="oh",
                                     bufs=4)
                        nc.vector.tensor_tensor(
                            out=oh[:],
                            in0=dcol.to_broadcast([P, WSL]),
                            in1=iotaWJ_s[:, j * WSL:(j + 1) * WSL],
                            op=mybir.AluOpType.is_equal)
                        pw = ps.tile([P, WSL], F32, name="pw", tag="pacc",
                                     space="PSUM", bufs=3)
                        nc.tensor.matmul(out=pw[:], lhsT=msg[csl],
                                         rhs=oh[:], start=True, stop=True)
                        a = acc[:, w * WSL:(w + 1) * WSL]
                        nc.vector.tensor_add(out=a, in0=a, in1=pw[:])

            def gather_phase_chunks(ph, dloc_s, ga_s, tbl_t):
                """Generator of scatter_chunks items for a gather phase.
                Gathers are emitted with lookahead 2 so that tile-pool buffer
                reuse (bufs=3) never outruns already-emitted consumers."""
                LA = 2
                icol = 0
                col0 = 0
                for g in range(NRES):
                    sched = ph["sched"][g]["chunks"]
                    nin = ph["n_instr"][g]
                    tiles = [None] * nin

                    def emit(ii, icol0=icol):
                        gt = sb.tile([P, GCH, H], BF16, name="gt", tag="gat")
                        gather(gt[:], tbl_t, g, ga_s, icol0 + ii)
                        tiles[ii] = gt
                    for ii in range(min(LA, nin)):
                        emit(ii)
                    for ii in range(nin):
                        if ii + LA < nin:
                            emit(ii + LA)
                        for c in range(GCH):
                            i = ii * GCH + c
                            wlist = sched[i]
                            if not wlist:
                                continue
                            yield (tiles[ii], np.s_[:, c, :],
                                   dloc_s[:, col0 + i:col0 + i + 1], wlist)
                    icol += nin
                    col0 += ph["nch_g"][g]

            # ---------------- init ----------------
            for bi in range(math.ceil(NB / GCH)):
                b0 = bi * GCH
                b1_ = min(NB, b0 + GCH)
                xei = sb.tile([7, GIDX_N], BF16, name="xei", tag="xet",
                              bufs=2)
                nc.sync.dma_start(
                    out=xei[:, :(b1_ - b0) * P],
                    in_=xeI_in[:, b0 * P:b1_ * P])
                for b in range(b0, b1_):
                    pi = ps.tile([P, H], F32, name="pi", tag="ptmp",
                                 space="PSUM")
                    nc.tensor.matmul(out=pi[:], lhsT=Wi7T_s[:],
                                     rhs=xei[:, (b - b0) * P:(b - b0 + 1) * P],
                                     start=True, stop=True)
                    nc.scalar.activation(out=hA_T[:, b * P:(b + 1) * P],
                                         in_=pi[:], func=_relu())

            # ---------------- message-passing iterations ----------------
            nloops = {"init": 0, "loop1": 1, "noscat": 1}.get(upto, depth)
            for it in range(nloops):
                # mA = relu(W_m h) -> bf16 table tile -> DRAM -> AllGather
                for b in range(NB):
                    hb = sb.tile([P, P], BF16, name="hb", tag="hb")
                    nc.vector.tensor_copy(out=hb[:],
                                          in_=hA_T[:, b * P:(b + 1) * P])
                    pm = ps.tile([P, H], F32, name="pm", tag="ptmp",
                                 space="PSUM")
                    nc.tensor.matmul(out=pm[:], lhsT=hb[:], rhs=WmT_s[:],
                                     start=True, stop=True)
                    nc.scalar.activation(out=tab_tile[:, b, :], in_=pm[:],
                                         func=_relu())
                nc.sync.dma_start(
                    out=loc[:, :].rearrange("(p b) h -> p (b h)", p=P),
                    in_=tab_tile[:])
                allgather(loc, tbls[it])
                if upto == "noscat":
                    for gi2 in gather_phase_chunks(plan.loop, dlocL_s, gaL_s,
                                                   tbls[it]):
                        pass
                else:
                    scatter_chunks(
                        gather_phase_chunks(plan.loop, dlocL_s, gaL_s,
                                            tbls[it]),
                        hA_T)

            # ---------------- final aggregation ----------------
            if upto in ("init", "loop1", "noscat", "loop"):
                stub = sb.tile([1, 1], F32, name="stub", tag="oc", bufs=2)
                nc.vector.tensor_copy(out=stub[:], in_=hA_T[0:1, 0:1])
                nc.sync.dma_start(out=out_ext[0:1, :], in_=stub[:])
                raise _Truncated()
            # h_final table (transposed state) -> DRAM -> AllGather
            for b in range(NB):
                pt = ps.tile([P, H], F32, name="pt", tag="ptmp", space="PSUM")
                nc.tensor.transpose(out=pt[:], in_=hA_T[:, b * P:(b + 1) * P],
                                    identity=ident_s[:])
                nc.vector.tensor_copy(out=tab_tile[:, b, :], in_=pt[:])
            nc.sync.dma_start(
                out=loc[:, :].rearrange("(p b) h -> p (b h)", p=P),
                in_=tab_tile[:])
            allgather(loc, tbls[depth])

            # stream 1: h0 recompute for edge ids >= N (overlaps AG + gathers)
            def f1_chunks():
                nbatch = plan.C1 // GCH
                for bi in range(nbatch):
                    xet = sb.tile([7, GIDX_N], BF16, name="xet", tag="xet",
                                  bufs=2)
                    nc.sync.dma_start(
                        out=xet[:],
                        in_=xeF_in[:, bi * GIDX_N:(bi + 1) * GIDX_N])
                    for c in range(GCH):
                        i = bi * GCH + c
                        wlist = plan.f1_sched[i]
                        if not wlist:
                            continue
                        ph0 = ps.tile([P, H], F32, name="ph0", tag="ptmp",
                                      space="PSUM")
                        nc.tensor.matmul(out=ph0[:],
                                         lhsT=xet[:, c * P:(c + 1) * P],
                                         rhs=Wi7T_s[:], start=True, stop=True)
                        msg = sb.tile([P, H], BF16, name="msg", tag="msg",
                                      bufs=4)
                        nc.scalar.activation(out=msg[:], in_=ph0[:],
                                             func=_relu())
                        yield (msg, np.s_[:, :],
                               dlocF1_s[:, i:i + 1], wlist)

            # emit: f0 gathers first (Pool/DMA), then f1 (PE) runs under them,
            # then f0 scatter consumes the gathered tiles.
            f0_iter = gather_phase_chunks(plan.f0, dlocF_s, gaF_s, tbls[depth])
            scatter_chunks(f1_chunks(), nacc_T)
            scatter_chunks(f0_iter, nacc_T)

            # ---------------- node_emb + pooling ----------------
            gps = ps.tile([P, GW], F32, name="gps", tag="gps", space="PSUM",
                          bufs=1)
            for b in range(NB):
                if b % GCH == 0:
                    b0 = b
                    b1_ = min(NB, b0 + GCH)
                    xtt = sb.tile([4, GIDX_N], BF16, name="xtt", tag="xet",
                                  bufs=2)
                    nc.sync.dma_start(
                        out=xtt[:, :(b1_ - b0) * P],
                        in_=xT_in[:, b0 * P:b1_ * P])
                nb16 = sb.tile([P, P], BF16, name="nb16", tag="hb")
                nc.vector.tensor_copy(out=nb16[:],
                                      in_=nacc_T[:, b * P:(b + 1) * P])
                p2 = ps.tile([P, H], F32, name="p2", tag="ptmp", space="PSUM")
                nc.tensor.matmul(out=p2[:], lhsT=nb16[:],
                                 rhs=WahT_s[:], start=True, stop=False)
                nc.tensor.matmul(out=p2[:],
                                 lhsT=xtt[:, (b - b0) * P:(b - b0 + 1) * P],
                                 rhs=WaxT_s[:], start=False, stop=True)
                ne2 = sb.tile([P, H], BF16, name="ne2", tag="msg", bufs=4)
                nc.scalar.activation(out=ne2[:], in_=p2[:], func=_relu())
                ohg = sb.tile([P, GW], BF16, name="ohg", tag="ohg")
                nc.vector.tensor_tensor(
                    out=ohg[:],
                    in0=batchloc_s[:, b:b + 1].to_broadcast([P, GW]),
                    in1=iotaG_s[:], op=mybir.AluOpType.is_equal)
                nc.tensor.matmul(out=gps[:], lhsT=ne2[:], rhs=ohg[:],
                                 start=(b == 0), stop=(b == NB - 1))

            tgw = sb.tile([P, GW], F32, name="tgw", tag="ohg")
            nc.vector.tensor_copy(out=tgw[:], in_=gps[:])
            nc.sync.dma_start(out=gwin_local[:, :], in_=tgw[:])
            allgather(gwin_local, gwin_all)
            nc.vector.memset(gfull[:], 0.0)
            for j in range(NCORES):
                wj = min(GW, G - plan.g_bases[j])
                tw = sb.tile([P, GW], F32, name="twj", tag="ohg")
                nc.sync.dma_start(out=tw[:], in_=gwin_all[j * P:(j + 1) * P, :])
                nc.vector.tensor_add(
                    out=gfull[:, plan.g_bases[j]:plan.g_bases[j] + wj],
                    in0=gfull[:, plan.g_bases[j]:plan.g_bases[j] + wj],
                    in1=tw[:, :wj])

            # ---------------- FFN (replicated on all cores) ----------------
            NGC = math.ceil(G / 512)
            for gc in range(NGC):
                g0, g1 = gc * 512, min((gc + 1) * 512, G)
                pz2 = ps.tile([P, 512], F32, name="pz2", tag="pz2",
                              space="PSUM", bufs=1)
                for f in range(4):
                    pz = ps.tile([P, 512], F32, name="pz", tag="ptmp",
                                 space="PSUM")
                    nc.tensor.matmul(out=pz[:, :g1 - g0],
                                     lhsT=W1T_s[:, f * P:(f + 1) * P],
                                     rhs=gfull[:, g0:g1], start=True, stop=True)
                    z1 = sb.tile([P, 512], F32, name="z1", tag="z1", bufs=2)
                    nc.scalar.activation(out=z1[:, :g1 - g0],
                                         in_=pz[:, :g1 - g0], func=_relu(),
                                         bias=b1r_s[:, f:f + 1])
                    nc.tensor.matmul(out=pz2[:, :g1 - g0], lhsT=W2T_f[f][:],
                                     rhs=z1[:, :g1 - g0], start=(f == 0),
                                     stop=(f == 3))
                z2 = sb.tile([P, 512], F32, name="z2", tag="z1", bufs=2)
                nc.vector.tensor_add(
                    out=z2[:, :g1 - g0], in0=pz2[:, :g1 - g0],
                    in1=b2r_s[:, 0:1].to_broadcast([P, g1 - g0]))
                po = ps.tile([1, 512], F32, name="po", tag="ptmp",
                             space="PSUM")
                nc.tensor.matmul(out=po[:, :g1 - g0], lhsT=WlastT_s[:],
                                 rhs=z2[:, :g1 - g0], start=True, stop=True)
                oc = sb.tile([1, 512], F32, name="oc", tag="oc", bufs=2)
                nc.vector.tensor_add(
                    out=oc[:, :g1 - g0], in0=po[:, :g1 - g0],
                    in1=blast_s[0:1, 0:1].to_broadcast([1, g1 - g0]))
                nc.sync.dma_start(out=out_ext[g0:g1, :], in_=oc[:, :g1 - g0])

      except _Truncated:
        pass
    nc.compile()
    if split:
        _split_excess_waits(nc)
    return nc


def _split_excess_waits(nc, max_waits=1):
    k = 0
    for f in nc.m.functions:
        for bb in f.blocks:
            new = []
            for ins in bb.instructions:
                si = ins.sync_info
                if si is not None and len(si.on_wait) > max_waits:
                    waits = list(si.on_wait)
                    for w in waits[:-max_waits]:
                        nop = mybir.InstNoOp(name=f"I-waitsplit-{k}",
                                             engine=ins.engine)
                        k += 1
                        nop.sync_info = mybir.SyncInfo(on_wait=[w],
                                                       on_update=[])
                        new.append(nop)
                    si.on_wait = waits[-max_waits:]
                new.append(ins)
            bb.instructions = new
    return k


# ----------------------------------------------------------------------------
# inputs
# ----------------------------------------------------------------------------

def _in_maps(plan, weights):
    com = {
        "WmT": np.ascontiguousarray(weights["W_m"].T).astype(BF16NP),
        "Wi7T": np.ascontiguousarray(weights["W_i"].T).astype(BF16NP),
        "WaxT": np.ascontiguousarray(weights["W_a"][:, :4].T).astype(BF16NP),
        "WahT": np.ascontiguousarray(weights["W_a"][:, 4:].T).astype(BF16NP),
        "W1T": np.ascontiguousarray(weights["W1"].T),
        "W2T": np.ascontiguousarray(weights["W2"].T),
        "WlastT": np.ascontiguousarray(weights["W_last"].T),
        "b1r": np.ascontiguousarray(weights["b1"].reshape(4, H).T),
        "b2r": weights["b2"].reshape(H, 1).copy(),
        "blast": weights["b_last"].reshape(1, 1).copy(),
        "iotaWJ": np.tile(np.arange(JMAX * WSL, dtype=np.float32), (P, 1)),
        "iotaG": np.tile(np.arange(plan.GW, dtype=np.float32), (P, 1)),
        "ident": np.eye(P, dtype=np.float32),
    }
    maps = []
    for k in range(NCORES):
        m = dict(com)
        m["xeI"] = plan.xe_init[k]
        m["xeF"] = plan.f1_xe[k]
        m["xT"] = plan.xT[k]
        m["batchloc"] = plan.batchloc[k]
        m["gaL"] = plan.loop["ga"][k]
        m["gaF"] = plan.f0["ga"][k]
        m["dlocL"] = plan.loop["dloc"][k]
        m["dlocF"] = plan.f0["dloc"][k]
        m["dlocF1"] = plan.f1_dloc[k]
        maps.append(m)
    return maps


def _prep_all(x, edge_index, edge_attr, batch, depth, weights, G):
    plan = _host_prep(np.asarray(x, np.float32), np.asarray(edge_index),
                      np.asarray(edge_attr, np.float32), np.asarray(batch),
                      int(depth), G)
    maps = _in_maps(plan, weights)
    return plan, maps


def kernel(x, edge_index, edge_attr, batch, depth,
           W_i, W_m, W_a, W1, b1, W2, b2, W_last, b_last):
    weights = {
        "W_i": np.asarray(W_i, np.float32), "W_m": np.asarray(W_m, np.float32),
        "W_a": np.asarray(W_a, np.float32), "W1": np.asarray(W1, np.float32),
        "b1": np.asarray(b1, np.float32), "W2": np.asarray(W2, np.float32),
        "b2": np.asarray(b2, np.float32),
        "W_last": np.asarray(W_last, np.float32),
        "b_last": np.asarray(b_last, np.float32),
    }
    G = 2048
    plan, maps = _prep_all(x, edge_index, edge_attr, batch, depth, weights, G)
    nc = _build(plan, split=True)
    res = run_bass_kernel_spmd(nc, maps, list(range(NCORES)))
    return np.asarray(res.results[0]["out"]).reshape(G, 1).astype(np.float32)
